# revision 4
# baseline (speedup 1.0000x reference)
import numpy as np

N = 16384; E = 262144; DIM = 3; WIN = 5; HID = 128; PT = 16; NTYPES = 9
G = 32; GG = G * G
FC = 32; LIFT = 32; PROJC = 32; NL = 2; M1 = 16; M2 = 16
NCORES = 8
P = 128
EPC = E // NCORES          # 32768 edges per core
TN = 512                   # moving free-dim per matmul
NT = EPC // TN             # 64 tiles per core
KCH = 3                    # 384 = 3 * 128 contraction chunks

PROFILE = False
LAST_EXEC_NS = 0

_NC_CACHE = {}


def _gelu(x):
    x = x.astype(np.float32)
    c = np.float32(np.sqrt(2.0 / np.pi))
    return np.float32(0.5) * x * (np.float32(1.0) + np.tanh(c * (x + np.float32(0.044715) * x * x * x)))


def _relu(x):
    return np.maximum(x, np.float32(0.0))


def _ln(x, g, be):
    mu = x.mean(-1, keepdims=True)
    v = ((x - mu) ** 2).mean(-1, keepdims=True)
    return (x - mu) / np.sqrt(v + np.float32(1e-5)) * g + be


def _mlp_np(x, p, act):
    Ws, bs = p['W'], p['b']
    for i in range(len(Ws)):
        x = x @ Ws[i] + bs[i]
        if i < len(Ws) - 1:
            x = act(x)
    if 'g' in p:
        x = _ln(x, p['g'], p['be'])
    return x


def _segsum(data, idx, nseg):
    # exact segment sum via float64 cumsum on stably-sorted rows
    order = np.argsort(idx, kind='stable')
    d = data[order].astype(np.float64)
    counts = np.bincount(idx, minlength=nseg)
    ends = np.cumsum(counts)
    starts = ends - counts
    cs = np.concatenate([np.zeros((1, d.shape[1]), np.float64), np.cumsum(d, axis=0)], axis=0)
    return (cs[ends] - cs[starts]).astype(np.float32)


def _to_np(v):
    if isinstance(v, dict):
        return {k: _to_np(x) for k, x in v.items()}
    if isinstance(v, (list, tuple)):
        return [_to_np(x) for x in v]
    return np.asarray(v)


def _build_edge_mlp_module():
    import concourse.bass as bass
    import concourse.tile as tile
    from concourse import mybir
    from concourse.bacc import Bacc

    nc = Bacc()
    f32 = mybir.dt.float32
    xt = nc.dram_tensor("xt", [KCH * P, EPC], f32, kind="ExternalInput")
    w1 = nc.dram_tensor("w1", [KCH * P, P], f32, kind="ExternalInput")
    w2 = nc.dram_tensor("w2", [P, P], f32, kind="ExternalInput")
    w3 = nc.dram_tensor("w3", [P, P], f32, kind="ExternalInput")
    bb = nc.dram_tensor("bb", [P, 3], f32, kind="ExternalInput")
    zt = nc.dram_tensor("zt", [P, EPC], f32, kind="ExternalOutput")

    with tile.TileContext(nc) as tc:
        with (
            tc.tile_pool(name="wp", bufs=1) as wp,
            tc.tile_pool(name="sb", bufs=3) as sb,
            tc.tile_pool(name="ps", bufs=2, space="PSUM") as ps,
        ):
            w1t = []
            for k in range(KCH):
                t = wp.tile([P, P], f32, tag=f"w1c{k}")
                nc.sync.dma_start(t[:], w1[k * P:(k + 1) * P, :])
                w1t.append(t)
            w2t = wp.tile([P, P], f32, tag="w2"); nc.sync.dma_start(w2t[:], w2[:])
            w3t = wp.tile([P, P], f32, tag="w3"); nc.sync.dma_start(w3t[:], w3[:])
            bt = wp.tile([P, 3], f32, tag="bb"); nc.sync.dma_start(bt[:], bb[:])

            for j in range(NT):
                sl = slice(j * TN, (j + 1) * TN)
                xk = []
                for k in range(KCH):
                    t = sb.tile([P, TN], f32, tag=f"x{k}")
                    nc.sync.dma_start(t[:], xt[k * P:(k + 1) * P, sl])
                    xk.append(t)
                p1 = ps.tile([P, TN], f32, tag="p1")
                for k in range(KCH):
                    nc.tensor.matmul(p1[:], lhsT=w1t[k][:], rhs=xk[k][:],
                                     start=(k == 0), stop=(k == KCH - 1))
                z1 = sb.tile([P, TN], f32, tag="z1")
                nc.scalar.activation(z1[:], p1[:], mybir.ActivationFunctionType.Relu, bias=bt[:, :1])
                p2 = ps.tile([P, TN], f32, tag="p2")
                nc.tensor.matmul(p2[:], lhsT=w2t[:], rhs=z1[:], start=True, stop=True)
                z2 = sb.tile([P, TN], f32, tag="z2")
                nc.scalar.activation(z2[:], p2[:], mybir.ActivationFunctionType.Relu, bias=bt[:, 1:2])
                p3 = ps.tile([P, TN], f32, tag="p3")
                nc.tensor.matmul(p3[:], lhsT=w3t[:], rhs=z2[:], start=True, stop=True)
                z3 = sb.tile([P, TN], f32, tag="z3")
                nc.vector.tensor_scalar_add(z3[:], p3[:], bt[:, 2:3])
                nc.sync.dma_start(zt[:, sl], z3[:])

    if not nc.is_finalized():
        nc.finalize()
    return nc


def _run_spmd(nc, in_maps):
    global LAST_EXEC_NS
    from concourse.bass_utils import run_bass_kernel_spmd
    if PROFILE:
        try:
            import tempfile
            from trn_agent_boot.trn_boot import _ntff_profile_via_ctypes
            from concourse import bass2jax, bass_utils
            import gauge.profiler
            from concourse._compat import FishPath
            hook = _ntff_profile_via_ctypes("/opt/axon/libaxon_pjrt.so")
            neff_dir = tempfile.mkdtemp()
            with hook(neff_dir, [0]):
                results = bass2jax.run_bass_via_pjrt(nc, in_maps, n_cores=NCORES)
            profile = gauge.profiler.Profile(
                profile_path=FishPath(neff_dir), kernel_dev_mode=True,
                profile_on_exit=False, bass_kernel=nc.m,
                offline_processing=True, fname="*_body*")
            pres = bass_utils._process_ntff_profile(
                profile, neff_dir, nc, list(range(NCORES)), None, False, {}, False)
            LAST_EXEC_NS += pres.exec_time_ns
            return results
        except Exception as e:
            print("profiled run failed, falling back:", repr(e))
    res = run_bass_kernel_spmd(nc, in_maps, list(range(NCORES)), trace=False)
    return res.results


def _edge_mlp_device(x, W1, b1v, W2, b2v, W3, b3v):
    """x [E,384] fp32 -> x@W1+b1 relu @W2+b2 relu @W3+b3  (pre-LN), on 8 cores."""
    if 'nc' not in _NC_CACHE:
        _NC_CACHE['nc'] = _build_edge_mlp_module()
    nc = _NC_CACHE['nc']
    xT = np.ascontiguousarray(x.T.astype(np.float32))  # [384, E]
    wm = {
        "w1": np.ascontiguousarray(W1.astype(np.float32)),
        "w2": np.ascontiguousarray(W2.astype(np.float32)),
        "w3": np.ascontiguousarray(W3.astype(np.float32)),
        "bb": np.ascontiguousarray(np.stack(
            [b1v, b2v, b3v], axis=1).astype(np.float32)),
    }
    in_maps = [{"xt": np.ascontiguousarray(xT[:, i * EPC:(i + 1) * EPC]), **wm}
               for i in range(NCORES)]
    results = _run_spmd(nc, in_maps)
    zT = np.concatenate([results[i]["zt"] for i in range(NCORES)], axis=1)  # [128, E]
    return np.ascontiguousarray(zT.T)


def kernel(x_type, pos, recent_pos, edge_index, edge_attr, node_dist,
           latent_queries, nbr_in_q, nbr_in_p, nbr_out_q, nbr_out_p, params):
    global LAST_EXEC_NS
    LAST_EXEC_NS = 0
    x_type = np.asarray(x_type); pos = np.asarray(pos, np.float32)
    recent_pos = np.asarray(recent_pos, np.float32)
    edge_index = np.asarray(edge_index); edge_attr = np.asarray(edge_attr, np.float32)
    node_dist = np.asarray(node_dist, np.float32)
    latent_queries = np.asarray(latent_queries, np.float32)
    nbr_in_q = np.asarray(nbr_in_q); nbr_in_p = np.asarray(nbr_in_p)
    nbr_out_q = np.asarray(nbr_out_q); nbr_out_p = np.asarray(nbr_out_p)
    params = _to_np(params)

    def edge_mlp384(xcat, p):
        """3-layer MLP with 384-dim input on device, LN on host."""
        try:
            z = _edge_mlp_device(xcat, p['W'][0], p['b'][0], p['W'][1], p['b'][1],
                                 p['W'][2], p['b'][2])
        except Exception as e:
            print("device edge MLP failed, host fallback:", repr(e))
            z = _relu(xcat @ p['W'][0] + p['b'][0])
            z = _relu(z @ p['W'][1] + p['b'][1])
            z = z @ p['W'][2] + p['b'][2]
        return _ln(z, p['g'], p['be'])

    # node / edge encoders
    nf = _mlp_np(np.concatenate([params['embed'][x_type], pos], -1),
                 params['node_in'], _relu)
    ef = _mlp_np(edge_attr, params['edge_in'], _relu)
    src, dst = edge_index[0], edge_index[1]

    def sch(p, nf, ef):
        xcat = np.concatenate([nf[dst], nf[src], ef], -1)  # [E, 384]
        m = edge_mlp384(xcat, p['edge'])
        m = m * node_dist[:, None]
        ag = _segsum(m, dst, N)
        nu = _mlp_np(np.concatenate([nf, ag], -1), p['node'], _relu)
        return nf + nu, ef + m

    nf, ef = sch(params['mp_in'], nf, ef)
    out = _mlp_np(nf, params['node_out'], _relu)  # [N,3]
    pos2d = np.tanh(recent_pos @ params['W2d'] + params['b2d']).astype(np.float32)

    # in-GNO
    kin = np.concatenate([pos2d[nbr_in_p] - latent_queries[nbr_in_q], out[nbr_in_p]], -1)
    kv = _mlp_np(kin, params['gno_in'], _gelu)
    s = _segsum(kv, nbr_in_q, GG)
    c = np.bincount(nbr_in_q, minlength=GG).astype(np.float32)[:, None]
    in_p = s / np.maximum(c, np.float32(1.0))
    x = in_p.reshape(G, G, DIM)[None].transpose(0, 3, 1, 2)  # [1,3,G,G]

    # FNO
    f = params['fno']

    def c11(x, W, b):
        return np.einsum('bchw,cd->bdhw', x, W).astype(np.float32) + b[None, :, None, None]

    x = c11(_gelu(c11(x, f['liftW1'], f['liftb1'])), f['liftW2'], f['liftb2'])

    def spec(x, blk):
        B, C, H, W = x.shape
        xf = np.fft.rfft2(x)
        w1 = blk['w1r'] + 1j * blk['w1i']
        w2 = blk['w2r'] + 1j * blk['w2i']
        o1 = np.einsum('bixy,ioxy->boxy', xf[:, :, :M1, :M2], w1)
        o2 = np.einsum('bixy,ioxy->boxy', xf[:, :, H - M1:, :M2], w2)
        of = np.zeros((B, C, H, xf.shape[-1]), np.complex128)
        of[:, :, :M1, :M2] = o1
        of[:, :, H - M1:, :M2] = o2
        return np.fft.irfft2(of, s=(H, W)).astype(np.float32)

    for blk in f['blocks']:
        h = _gelu(x)
        x = spec(np.tanh(h), blk) + c11(h, blk['skW'], blk['skb'])
        h = _gelu(x)
        x = c11(_gelu(c11(h, blk['m1W'], blk['m1b'])), blk['m2W'], blk['m2b']) \
            + c11(h, blk['msW'], blk['msb'])
    lat = c11(_gelu(c11(x, f['pW1'], f['pb1'])), f['pW2'], f['pb2'])  # [1,HID,G,G]
    lat = lat.reshape(GG, HID)

    # out-GNO
    kout = _mlp_np(latent_queries[nbr_out_p] - pos2d[nbr_out_q], params['gno_out'], _gelu)
    v = kout * lat[nbr_out_p]
    s2 = _segsum(v, nbr_out_q, N)
    c2 = np.bincount(nbr_out_q, minlength=N).astype(np.float32)[:, None]
    o = s2 / np.maximum(c2, np.float32(1.0))

    nf2 = (o @ params['projW'] + params['projb']).astype(np.float32)
    nf2, ef = sch(params['mp_out'], nf2, ef)
    return _mlp_np(nf2, params['node_out'], _relu).astype(np.float32)
